# revision 96
# baseline (speedup 1.0000x reference)
"""MoE routing kernel for Trainium2 (8 NeuronCores, batch-parallel).

Problem: nn_MoE_47278999994656.
  x [8, 256, 80, 80] f32 + gate Linear(256->5) + 5 experts
  (residual conv1x1 on each 128-ch half, gated by a sigmoid transform),
  top-1 masked-softmax gate => weights are EXACTLY one-hot, so
  out[b] = expert_{argmax_e logits[b,e]}(x[b]).

Sharding: data-parallel over batch, core i computes batch item i.

Per core:
  - x transfers as bf16 (host-cast; device compute was already bf16) on
    three DMA queues (sync/scalar/gpsimd), sized to each queue's
    measured rate; the fused expert table rides the queue tails.
  - The gate runs as PSUM-accumulated bf16 matmuls chunk-by-chunk under
    the load; junk matmuls bridge the select window so the PE clock
    p-state stays ramped into phase 2.
  - Expert select: argmax index (iota dot one-hot mask, broadcast via a
    1-col matmul) drives ONE gpsimd ap_gather (48 wrapped row-indices)
    over a fused [128, 165, 16] table holding (I+W)^T blocks, the folded
    H weights, the replicated Wt2 block, and the bias row.
  - The H layer is algebraically folded: H = (Wt1(I+W))@x + (bt1+Wt1 b),
    so D and H matmuls both read x directly (no D->H dependency).
  - A (sigmoid arg) is 128-replicated via the Wt2-replication matmul into
    a paired 2-bank PSUM tile; ONE sigmoid covers both halves.
  - Combine: two fused scalar_tensor_tensor ops on Vector
    ((D_psum + b) * s, straight from PSUM), final add on GpSimd (SBUF
    bf16), paired output chunks written bf16 on alternating queues and
    upcast on host.
  - Phase 2 is a single-chunk software pipeline; the S queue alternates
    [sig(k), relu(k+2)] so the A-matmul round-trip latency hides under
    the previous sigmoid. PSUM: dps2 + hps2 + aps2x2banks = 8 banks.
"""

import numpy as np

import concourse.bacc as bacc_mod
import concourse.bass as bass
import concourse.mybir as mybir
import concourse.tile as tile
from concourse.bass_utils import run_bass_kernel_spmd

B, C, H, W = 8, 256, 80, 80
HW = H * W          # 6400
HALF = 128
QUARTER = 64
E = 5
NCORES = 8

# expert-layer chunks: 12 x 512 + 1 x 256 (psum bank holds 512 f32)
CHUNKS = [(i * 512, 512) for i in range(12)] + [(6144, 256)]
# input DMA chunks per half-half (x split across 4 DMA queues)
DCH = [(0, 1600), (1600, 1600)]      # per queue: 2 chunks of 1600 cols

# Expert tables are gathered with gpsimd ap_gather (16 row-indices of
# 16-wide rows per gather -> 256 contiguous elements). Table layout
# [128, 5*16, 16]: rows 16e..16e+15 hold expert e's 256-wide payload.
#   uD payload:  (I+Wrgb_e)^T [0:128] | (I+Wtir_e)^T [128:256]
#   uHA payload: Vrgb_e^T [0:64] | Vtir_e^T [64:128] | Wt2 rep [128:256]
#   bias payload: b_rgb, b_tir, c_stack, bt2 at cols 0:4 (bf16, upconverted)
NROW = 16 * E

N_JUNK_A = 5        # PE bridge: gate end -> index broadcast matmul
N_JUNK_B = 9        # PE bridge: index matmul -> first D matmul
N_JUNK_MID = 5      # PE filler per DMA chunk boundary (p-state hold)

F32 = mybir.dt.float32
BF16 = mybir.dt.bfloat16
U16 = mybir.dt.uint16
AX = mybir.AxisListType.X
ALU = mybir.AluOpType
AF = mybir.ActivationFunctionType


def build_nc() -> bass.Bass:
    nc = bacc_mod.Bacc()

    x0_d = nc.dram_tensor("x0", [HALF, HW], BF16, kind="ExternalInput")
    x1_d = nc.dram_tensor("x1", [HALF, HW], BF16, kind="ExternalInput")
    # one fused table: rows 0:80 = uD, 80:160 = uHA, 160:165 = bias row e
    u3_d = nc.dram_tensor("u3", [HALF, 165, 16], BF16, kind="ExternalInput")
    poff_d = nc.dram_tensor("poff", [HALF, 3], F32, kind="ExternalInput")
    wg_d = nc.dram_tensor("wg", [HALF, 2, E], BF16, kind="ExternalInput")
    bg_d = nc.dram_tensor("bg", [1, E], F32, kind="ExternalInput")
    iv_d = nc.dram_tensor("iv", [1, E], F32, kind="ExternalInput")
    out_d = nc.dram_tensor("out", [HALF, HW], BF16, kind="ExternalOutput")

    with tile.TileContext(nc) as tc:
        with (
            tc.tile_pool(name="big", bufs=1) as big,
            tc.tile_pool(name="const", bufs=1) as const,
            tc.tile_pool(name="small", bufs=1) as small,
            tc.tile_pool(name="hsbp", bufs=6) as hsbp,
            tc.tile_pool(name="sstp", bufs=6) as sstp,
            tc.tile_pool(name="combp", bufs=6) as combp,
        ):
            # ---- persistent SBUF ----
            xb = big.tile([HALF, 2, HW], BF16)       # 25.6 KB/part
            u3_all = const.tile([HALF, 165, 16], BF16)
            poff = const.tile([HALF, 3], F32)
            wg = const.tile([HALF, 2, E], BF16)
            bgx = const.tile([1, E], F32)
            iv = const.tile([1, E], F32)

            t32a = small.tile([32, 32], F32)
            t32b = small.tile([32, 32], F32)
            ones1 = small.tile([1, HALF], F32)
            l51 = small.tile([E, 1], F32)
            lrow = small.tile([1, E], F32)
            lmax = small.tile([1, 1], F32)
            mrow = small.tile([1, E], F32)
            mi = small.tile([1, E], F32)
            idxf = small.tile([1, 1], F32)
            idx16 = small.tile([HALF, 3], mybir.dt.int16)
            gdum = small.tile([1, E], F32)
            gdum2 = small.tile([HALF, 2], mybir.dt.int16)
            usel3 = small.tile([HALF, 768], BF16)
            bself = small.tile([HALF, 4], F32)
            sdum = small.tile([1, 1], F32)

            # small gate/select constants first on the gpsimd queue
            nc.gpsimd.dma_start(out=wg[:], in_=wg_d[:])
            nc.gpsimd.dma_start(out=bgx[:], in_=bg_d[:])
            nc.gpsimd.dma_start(out=iv[:], in_=iv_d[:])
            nc.gpsimd.dma_start(out=poff[:], in_=poff_d[:])

            # dep-free init
            nc.vector.memset(t32a, 0.0)
            nc.vector.memset(ones1, 1.0)

            with (
                tc.tile_pool(name="gps", bufs=1, space="PSUM") as gps,
                tc.tile_pool(name="jps", bufs=1, space="PSUM") as jps,
            ):
                # ---- phase 1: x load on 3 DMA queues + gate under it ----
                for off, n in [(0, 3200), (3200, 1600)]:
                    nc.sync.dma_start(
                        out=xb[:, 0, off : off + n], in_=x0_d[:, off : off + n]
                    )
                    nc.scalar.dma_start(
                        out=xb[:, 1, off : off + n], in_=x1_d[:, off : off + n]
                    )
                nc.gpsimd.dma_start(
                    out=xb[:, 0, 4800:6400], in_=x0_d[:, 4800:6400]
                )
                nc.gpsimd.dma_start(
                    out=xb[:, 1, 4800:6400], in_=x1_d[:, 4800:6400]
                )
                # fused expert table last (needed only at gather time)
                nc.sync.dma_start(out=u3_all[:, 0:83, :], in_=u3_d[:, 0:83, :])
                nc.scalar.dma_start(
                    out=u3_all[:, 83:165, :], in_=u3_d[:, 83:165, :]
                )
                # pin the sigmoid act-func table AFTER the S-queue descs
                # (the implicit table loads otherwise delay the x1 load)
                nc.scalar.activation(
                    out=sdum, in_=ones1[0:1, 0:1], func=AF.Sigmoid
                )

                # PE warm-up burst on wg (lands ~5us before the first x
                # chunk): ramps the clock p-state so the gate matmuls run
                # at full speed and keep pace with the DMA
                for j in range(40):
                    jnk = jps.tile([E, 2 * E], F32, tag="j")
                    nc.tensor.matmul(
                        jnk, lhsT=wg[:, 0, :], rhs=wg[:, :, :]
                    )

                # 512-col gate sub-chunks; Tile waits per-region on the DMAs
                yg = gps.tile([E, 512], F32, tag="g")
                nmm = 2 * len(CHUNKS)
                k = 0
                for o, m in CHUNKS:
                    for h in range(2):
                        nc.tensor.matmul(
                            yg[:, 0:m],
                            lhsT=wg[:, h, :],
                            rhs=xb[:, h, o : o + m],
                            start=(k == 0),
                            stop=(k == nmm - 1),
                        )
                        k += 1

                # junk bridge A: gate end -> index broadcast matmul
                for j in range(N_JUNK_A):
                    jnk = jps.tile([E, 512], F32, tag="j")
                    nc.tensor.matmul(
                        jnk, lhsT=wg[:, 0, :],
                        rhs=xb[:, 0, (j % 8) * 512 : (j % 8) * 512 + 512],
                    )

                # ---- gate finalize -> argmax index (V-engine chain) ----
                nc.vector.reduce_sum(l51, yg, axis=AX)
                nc.vector.tensor_copy(t32a[0:E, 0:1], l51)
                nc.vector.transpose(t32b, t32a)
                nc.vector.tensor_add(lrow, t32b[0:1, 0:E], bgx[0:1, :])
                nc.vector.reduce_max(lmax, lrow, axis=AX)
                nc.vector.tensor_scalar(
                    out=mrow, in0=lrow, scalar1=lmax, scalar2=None,
                    op0=ALU.is_equal,
                )
                nc.vector.tensor_mul(mi, mrow, iv)   # iv = 16*e (row index)
                nc.vector.reduce_sum(idxf, mi, axis=AX)
                ibc = gps.tile([HALF, 1], F32, tag="g")
                nc.tensor.matmul(ibc, lhsT=ones1, rhs=idxf)
                # wrapped per-16-partition row indices: partition p holds
                # [16e + p%16, 80 + 16e + p%16, 160 + e]
                nc.vector.tensor_scalar(
                    out=idx16[:, 0:2], in0=poff[:, 0:2],
                    scalar1=ibc[:, 0:1], scalar2=None, op0=ALU.add,
                )
                nc.vector.tensor_scalar(
                    out=idx16[:, 2:3], in0=poff[:, 2:3],
                    scalar1=ibc[:, 0:1], scalar2=0.0625,
                    op0=ALU.add, op1=ALU.mult,
                )

                # gpsimd wake-up: a dependent no-op so the engine is
                # actively polling when idx16 lands
                nc.gpsimd.tensor_copy(gdum, mrow)
                # ---- select expert: ONE gpsimd ap_gather (48 idxs) ----
                nc.gpsimd.ap_gather(
                    usel3, u3_all, idx16,
                    channels=HALF, num_elems=165, d=16, num_idxs=48,
                )
                nc.vector.tensor_copy(bself, usel3[:, 512:516])

                # junk bridge B: index matmul -> first D matmul
                for j in range(N_JUNK_B):
                    jnk = jps.tile([E, 512], F32, tag="j")
                    nc.tensor.matmul(
                        jnk, lhsT=wg[:, 1, :],
                        rhs=xb[:, 1, (j % 8) * 512 : (j % 8) * 512 + 512],
                    )

            uDr = usel3[:, 0:HALF]           # (I+Wrgb)^T
            uDt = usel3[:, HALF : 2 * HALF]  # (I+Wtir)^T
            uVr = usel3[:, 256 : 256 + QUARTER]
            uVt = usel3[:, 256 + QUARTER : 256 + HALF]
            uW2 = usel3[:, 384:512]          # Wt2 replicated
            bsel = bself

            # ---- phase 2: selected expert, software-pipelined chunks ----
            with (
                tc.tile_pool(name="dps", bufs=2, space="PSUM") as dps,
                tc.tile_pool(name="hps", bufs=2, space="PSUM") as hps,
                tc.tile_pool(name="aps", bufs=2, space="PSUM") as aps,
            ):
                nch = len(CHUNKS)
                hp = [None] * nch
                hsb = [None] * nch

                def emit_h(ci):
                    off, n = CHUNKS[ci]
                    hp[ci] = hps.tile([HALF, 512], F32, tag="h", name="hp")
                    nc.tensor.matmul(
                        hp[ci][0:QUARTER, 0:n],
                        lhsT=uVr,
                        rhs=xb[:, 0, off : off + n],
                    )
                    nc.tensor.matmul(
                        hp[ci][QUARTER:HALF, 0:n],
                        lhsT=uVt,
                        rhs=xb[:, 1, off : off + n],
                        tile_position=(0, QUARTER),
                    )

                def emit_sig(ci):
                    # A layer: both halves into one paired 2-bank PSUM tile,
                    # then ONE sigmoid over both halves (S)
                    off, n = CHUNKS[ci]
                    a2 = aps.tile([HALF, 2, 512], F32, tag="a", name="a2")
                    nc.tensor.matmul(
                        a2[:, 0, 0:n],
                        lhsT=uW2[0:QUARTER, :],
                        rhs=hsb[ci][0:QUARTER, 0:n],
                        tile_position=(0, 0),
                    )
                    nc.tensor.matmul(
                        a2[:, 1, 0:n],
                        lhsT=uW2[QUARTER:HALF, :],
                        rhs=hsb[ci][QUARTER:HALF, 0:n],
                        tile_position=(QUARTER, 0),
                    )
                    sst = sstp.tile([HALF, 2, 512], BF16, tag="s", name="sst")
                    nc.scalar.activation(
                        out=sst[:, :, 0:n], in_=a2[:, :, 0:n],
                        func=AF.Sigmoid, bias=bsel[:, 3:4],
                    )
                    return sst

                def emit_comb(ci, sst, ocp, j, vadd=False):
                    # D late (short PSUM residency); (D+b)*s fused on V;
                    # final add on G into the paired out tile
                    off, n = CHUNKS[ci]
                    dr = dps.tile([HALF, 512], F32, tag="d", name="dr")
                    nc.tensor.matmul(
                        dr[:, 0:n], lhsT=uDr, rhs=xb[:, 0, off : off + n]
                    )
                    dt = dps.tile([HALF, 512], F32, tag="d", name="dt")
                    nc.tensor.matmul(
                        dt[:, 0:n], lhsT=uDt, rhs=xb[:, 1, off : off + n]
                    )
                    prt = combp.tile([HALF, 512], BF16, tag="p", name="prt")
                    nc.vector.scalar_tensor_tensor(
                        out=prt[:, 0:n], in0=dr[:, 0:n], scalar=bsel[:, 0:1],
                        in1=sst[:, 0, 0:n], op0=ALU.add, op1=ALU.mult,
                    )
                    ob = combp.tile([HALF, 512], BF16, tag="o", name="ob")
                    nc.vector.scalar_tensor_tensor(
                        out=ob[:, 0:n], in0=dt[:, 0:n], scalar=bsel[:, 1:2],
                        in1=sst[:, 1, 0:n], op0=ALU.add, op1=ALU.mult,
                    )
                    eng = nc.vector if vadd else nc.gpsimd
                    eng.tensor_add(ocp[:, j, 0:n], prt[:, 0:n], ob[:, 0:n])

                def emit_relu(ci):
                    off, n = CHUNKS[ci]
                    hsb[ci] = hsbp.tile(
                        [HALF, 512], BF16, tag="hsb", name="hsb"
                    )
                    nc.scalar.activation(
                        out=hsb[ci][:, 0:n], in_=hp[ci][:, 0:n],
                        func=AF.Relu, bias=bsel[:, 2:3],
                    )

                # single-chunk software pipeline; S queue alternates
                # [sig(k), relu(k+2)] so the A-matmul round-trip latency
                # hides under the previous sigmoid
                emit_h(0)
                if nch > 1:
                    emit_h(1)
                emit_relu(0)
                if nch > 1:
                    emit_relu(1)
                ocp = None
                for k in range(nch):
                    sst = emit_sig(k)
                    if k + 2 < nch:
                        emit_h(k + 2)
                        emit_relu(k + 2)
                    if k % 2 == 0:
                        ocp = combp.tile(
                            [HALF, 2, 512], BF16, tag="c", name="ocp"
                        )
                    emit_comb(k, sst, ocp, k % 2)
                    if k % 2 == 1 or k == nch - 1:
                        base = k - (k % 2)
                        off0 = CHUNKS[base][0]
                        ntot = sum(CHUNKS[c][1] for c in range(base, k + 1))
                        oq = nc.sync if (base // 2) % 2 == 0 else nc.scalar
                        oq.dma_start(
                            out=out_d[:, off0 : off0 + ntot],
                            in_=ocp[:, 0:2, 0:512]
                            if ntot == 1024
                            else ocp[:, 0, 0:ntot],
                        )

    nc.compile()
    return nc


def _pack_inputs(x, Wg, bg, Wrgb, brgb, Wtir, btir, Wt1, bt1, Wt2, bt2):
    import ml_dtypes
    eye = np.eye(HALF, dtype=np.float32)
    # per-expert 256-wide payloads, then sliced into 16 rows of 16
    udp = np.zeros((E, HALF, 256), dtype=np.float32)
    uhp = np.zeros((E, HALF, 256), dtype=np.float32)
    for e in range(E):
        Ar = Wrgb[e] + eye                      # [o, c]
        At = Wtir[e] + eye
        udp[e, :, 0:HALF] = Ar.T
        udp[e, :, HALF:] = At.T
        uhp[e, :, 0:QUARTER] = (Wt1[e] @ Ar).T
        uhp[e, :, QUARTER:HALF] = (Wt1[e] @ At).T
        rep = np.repeat(Wt2[e, 0][:, None], HALF, axis=1)   # [64, 128]
        uhp[e, 0:QUARTER, HALF:] = rep
        uhp[e, QUARTER:HALF, HALF:] = rep

    def to_table(p):
        # [E, 128, 256] -> [128, 16E, 16]: row 16e+j = payload cols 16j:16j+16
        t = p.reshape(E, HALF, 16, 16).transpose(1, 0, 2, 3).reshape(
            HALF, 16 * E, 16
        )
        return np.ascontiguousarray(t).astype(ml_dtypes.bfloat16)

    u3t = np.concatenate([to_table(udp), to_table(uhp)], axis=1)

    # bias rows 160+e of the fused table: [b_rgb, b_tir, c_stack, bt2, 0..]
    bias_t = np.zeros((HALF, E, 16), dtype=np.float32)
    for e in range(E):
        bias_t[:, e, 0] = brgb[e]
        bias_t[:, e, 1] = btir[e]
        bias_t[0:QUARTER, e, 2] = bt1[e] + Wt1[e] @ brgb[e]
        bias_t[QUARTER:HALF, e, 2] = bt1[e] + Wt1[e] @ btir[e]
        bias_t[:, e, 3] = bt2[e, 0]
    u3 = np.ascontiguousarray(np.concatenate(
        [u3t, bias_t.astype(ml_dtypes.bfloat16)], axis=1
    ))

    wgt = Wg.T.astype(np.float32)                   # [256, 5]
    wg_p = np.ascontiguousarray(
        np.stack([wgt[:HALF], wgt[HALF:]], axis=1)
    ).astype(ml_dtypes.bfloat16)                    # [128, 2, 5]
    bgx = np.ascontiguousarray((bg * float(HW))[None, :].astype(np.float32))
    # first table-row index of expert e
    ivx = (16.0 * np.arange(E, dtype=np.float32))[None, :]
    # wrapped per-16-partition row indices: partition p gathers rows
    # 16e+p%16 (uD), 80+16e+p%16 (uHA), 160+e (bias; (2560+16e)/16)
    pmod = np.arange(HALF, dtype=np.float32) % 16
    poffx = np.stack(
        [pmod, pmod + 16.0 * E, np.full(HALF, 2560.0, np.float32)], axis=1
    ).copy()

    common = {"u3": u3, "wg": wg_p, "bg": bgx, "iv": ivx, "poff": poffx}
    in_maps = []
    for b in range(B):
        m = dict(common)
        xr = x[b].reshape(C, HW)
        m["x0"] = np.ascontiguousarray(xr[:HALF]).astype(ml_dtypes.bfloat16)
        m["x1"] = np.ascontiguousarray(xr[HALF:]).astype(ml_dtypes.bfloat16)
        in_maps.append(m)
    return in_maps


_NC_CACHE = {}


def _get_nc():
    if "nc" not in _NC_CACHE:
        _NC_CACHE["nc"] = build_nc()
    return _NC_CACHE["nc"]


def kernel(x, Wg, bg, Wrgb, brgb, Wtir, btir, Wt1, bt1, Wt2, bt2, **run_kw):
    nc = _get_nc()
    in_maps = _pack_inputs(
        np.asarray(x), np.asarray(Wg), np.asarray(bg), np.asarray(Wrgb),
        np.asarray(brgb), np.asarray(Wtir), np.asarray(btir),
        np.asarray(Wt1), np.asarray(bt1), np.asarray(Wt2), np.asarray(bt2),
    )
    res = run_bass_kernel_spmd(nc, in_maps, core_ids=list(range(NCORES)), **run_kw)
    out = np.stack(
        [np.asarray(r["out"]).astype(np.float32) for r in res.results], axis=0
    )
    if run_kw:
        kernel.last_results = res
    return out.reshape(B, HALF, H, W)


# revision 97
# speedup vs baseline: 1.0242x; 1.0242x over previous
"""MoE routing kernel for Trainium2 (8 NeuronCores, batch-parallel).

Problem: nn_MoE_47278999994656.
  x [8, 256, 80, 80] f32 + gate Linear(256->5) + 5 experts
  (residual conv1x1 on each 128-ch half, gated by a sigmoid transform),
  top-1 masked-softmax gate => weights are EXACTLY one-hot, so
  out[b] = expert_{argmax_e logits[b,e]}(x[b]).

Sharding: data-parallel over batch, core i computes batch item i.

Per core:
  - x transfers as bf16 (host-cast; device compute was already bf16) on
    three DMA queues (sync/scalar/gpsimd), sized to each queue's
    measured rate; the fused expert table rides the queue tails.
  - The gate runs as PSUM-accumulated bf16 matmuls chunk-by-chunk under
    the load; junk matmuls bridge the select window so the PE clock
    p-state stays ramped into phase 2.
  - Expert select: argmax index (iota dot one-hot mask, broadcast via a
    1-col matmul) drives ONE gpsimd ap_gather (48 wrapped row-indices)
    over a fused [128, 165, 16] table holding (I+W)^T blocks, the folded
    H weights, the replicated Wt2 block, and the bias row.
  - The H layer is algebraically folded: H = (Wt1(I+W))@x + (bt1+Wt1 b),
    so D and H matmuls both read x directly (no D->H dependency).
  - A (sigmoid arg) is 128-replicated via the Wt2-replication matmul into
    a paired 2-bank PSUM tile; ONE sigmoid covers both halves.
  - Combine: two fused scalar_tensor_tensor ops on Vector
    ((D_psum + b) * s, straight from PSUM), final add on GpSimd (SBUF
    bf16), paired output chunks written bf16 on alternating queues and
    upcast on host.
  - Phase 2 is a single-chunk software pipeline; the S queue alternates
    [sig(k), relu(k+2)] so the A-matmul round-trip latency hides under
    the previous sigmoid. PSUM: dps2 + hps2 + aps2x2banks = 8 banks.
"""

import numpy as np

import concourse.bacc as bacc_mod
import concourse.bass as bass
import concourse.mybir as mybir
import concourse.tile as tile
from concourse.bass_utils import run_bass_kernel_spmd

B, C, H, W = 8, 256, 80, 80
HW = H * W          # 6400
HALF = 128
QUARTER = 64
E = 5
NCORES = 8

# expert-layer chunks: 12 x 512 + 1 x 256 (psum bank holds 512 f32)
CHUNKS = [(i * 512, 512) for i in range(12)] + [(6144, 256)]
# input DMA chunks per half-half (x split across 4 DMA queues)
DCH = [(0, 1600), (1600, 1600)]      # per queue: 2 chunks of 1600 cols

# Expert tables are gathered with gpsimd ap_gather (16 row-indices of
# 16-wide rows per gather -> 256 contiguous elements). Table layout
# [128, 5*16, 16]: rows 16e..16e+15 hold expert e's 256-wide payload.
#   uD payload:  (I+Wrgb_e)^T [0:128] | (I+Wtir_e)^T [128:256]
#   uHA payload: Vrgb_e^T [0:64] | Vtir_e^T [64:128] | Wt2 rep [128:256]
#   bias payload: b_rgb, b_tir, c_stack, bt2 at cols 0:4 (bf16, upconverted)
NROW = 16 * E

N_JUNK_A = 5        # PE bridge: gate end -> index broadcast matmul
N_JUNK_B = 9        # PE bridge: index matmul -> first D matmul
N_JUNK_MID = 5      # PE filler per DMA chunk boundary (p-state hold)

F32 = mybir.dt.float32
BF16 = mybir.dt.bfloat16
U16 = mybir.dt.uint16
AX = mybir.AxisListType.X
ALU = mybir.AluOpType
AF = mybir.ActivationFunctionType


def build_nc() -> bass.Bass:
    nc = bacc_mod.Bacc()

    x0_d = nc.dram_tensor("x0", [HALF, HW], BF16, kind="ExternalInput")
    x1_d = nc.dram_tensor("x1", [HALF, HW], BF16, kind="ExternalInput")
    # one fused table: rows 0:80 = uD, 80:160 = uHA, 160:165 = bias row e
    u3_d = nc.dram_tensor("u3", [HALF, 165, 16], BF16, kind="ExternalInput")
    poff_d = nc.dram_tensor("poff", [HALF, 3], F32, kind="ExternalInput")
    wg_d = nc.dram_tensor("wg", [HALF, 2, E], BF16, kind="ExternalInput")
    bg_d = nc.dram_tensor("bg", [1, E], F32, kind="ExternalInput")
    iv_d = nc.dram_tensor("iv", [1, E], F32, kind="ExternalInput")
    out_d = nc.dram_tensor("out", [HALF, HW], BF16, kind="ExternalOutput")

    with tile.TileContext(nc) as tc:
        with (
            tc.tile_pool(name="big", bufs=1) as big,
            tc.tile_pool(name="const", bufs=1) as const,
            tc.tile_pool(name="small", bufs=1) as small,
            tc.tile_pool(name="hsbp", bufs=6) as hsbp,
            tc.tile_pool(name="sstp", bufs=6) as sstp,
            tc.tile_pool(name="combp", bufs=6) as combp,
        ):
            # ---- persistent SBUF ----
            xb = big.tile([HALF, 2, HW], BF16)       # 25.6 KB/part
            u3_all = const.tile([HALF, 165, 16], BF16)
            poff = const.tile([HALF, 3], F32)
            wg = const.tile([HALF, 2, E], BF16)
            bgx = const.tile([1, E], F32)
            iv = const.tile([1, E], F32)

            t32a = small.tile([32, 32], F32)
            t32b = small.tile([32, 32], F32)
            ones1 = small.tile([1, HALF], F32)
            l51 = small.tile([E, 1], F32)
            lrow = small.tile([1, E], F32)
            lmax = small.tile([1, 1], F32)
            mrow = small.tile([1, E], F32)
            mi = small.tile([1, E], F32)
            idxf = small.tile([1, 1], F32)
            idx16 = small.tile([HALF, 3], mybir.dt.int16)
            gdum = small.tile([1, E], F32)
            gdum2 = small.tile([HALF, 2], mybir.dt.int16)
            usel3 = small.tile([HALF, 768], BF16)
            bself = small.tile([HALF, 4], F32)
            sdum = small.tile([1, 1], F32)

            # small gate/select constants first on the gpsimd queue
            nc.gpsimd.dma_start(out=wg[:], in_=wg_d[:])
            nc.gpsimd.dma_start(out=bgx[:], in_=bg_d[:])
            nc.gpsimd.dma_start(out=iv[:], in_=iv_d[:])
            nc.gpsimd.dma_start(out=poff[:], in_=poff_d[:])

            # dep-free init
            nc.vector.memset(t32a, 0.0)
            nc.vector.memset(ones1, 1.0)

            with (
                tc.tile_pool(name="gps", bufs=1, space="PSUM") as gps,
                tc.tile_pool(name="jps", bufs=1, space="PSUM") as jps,
            ):
                # ---- phase 1: x load on 3 DMA queues + gate under it ----
                for off, n in [(0, 3200), (3200, 1600)]:
                    nc.sync.dma_start(
                        out=xb[:, 0, off : off + n], in_=x0_d[:, off : off + n]
                    )
                    nc.scalar.dma_start(
                        out=xb[:, 1, off : off + n], in_=x1_d[:, off : off + n]
                    )
                nc.gpsimd.dma_start(
                    out=xb[:, 0, 4800:6400], in_=x0_d[:, 4800:6400]
                )
                nc.gpsimd.dma_start(
                    out=xb[:, 1, 4800:6400], in_=x1_d[:, 4800:6400]
                )
                # fused expert table last (needed only at gather time)
                nc.sync.dma_start(out=u3_all[:, 0:83, :], in_=u3_d[:, 0:83, :])
                nc.scalar.dma_start(
                    out=u3_all[:, 83:165, :], in_=u3_d[:, 83:165, :]
                )
                # pin the sigmoid act-func table AFTER the S-queue descs
                # (the implicit table loads otherwise delay the x1 load)
                nc.scalar.activation(
                    out=sdum, in_=ones1[0:1, 0:1], func=AF.Sigmoid
                )

                # 512-col gate sub-chunks; Tile waits per-region on the DMAs
                yg = gps.tile([E, 512], F32, tag="g")
                nmm = 2 * len(CHUNKS)
                k = 0
                for o, m in CHUNKS:
                    for h in range(2):
                        nc.tensor.matmul(
                            yg[:, 0:m],
                            lhsT=wg[:, h, :],
                            rhs=xb[:, h, o : o + m],
                            start=(k == 0),
                            stop=(k == nmm - 1),
                        )
                        k += 1

                # junk bridge A: gate end -> index broadcast matmul
                for j in range(N_JUNK_A):
                    jnk = jps.tile([E, 512], F32, tag="j")
                    nc.tensor.matmul(
                        jnk, lhsT=wg[:, 0, :],
                        rhs=xb[:, 0, (j % 8) * 512 : (j % 8) * 512 + 512],
                    )

                # ---- gate finalize -> argmax index (V-engine chain) ----
                nc.vector.reduce_sum(l51, yg, axis=AX)
                nc.vector.tensor_copy(t32a[0:E, 0:1], l51)
                nc.vector.transpose(t32b, t32a)
                nc.vector.tensor_add(lrow, t32b[0:1, 0:E], bgx[0:1, :])
                nc.vector.reduce_max(lmax, lrow, axis=AX)
                nc.vector.tensor_scalar(
                    out=mrow, in0=lrow, scalar1=lmax, scalar2=None,
                    op0=ALU.is_equal,
                )
                nc.vector.tensor_mul(mi, mrow, iv)   # iv = 16*e (row index)
                nc.vector.reduce_sum(idxf, mi, axis=AX)
                ibc = gps.tile([HALF, 1], F32, tag="g")
                nc.tensor.matmul(ibc, lhsT=ones1, rhs=idxf)
                # wrapped per-16-partition row indices: partition p holds
                # [16e + p%16, 80 + 16e + p%16, 160 + e]
                nc.vector.tensor_scalar(
                    out=idx16[:, 0:2], in0=poff[:, 0:2],
                    scalar1=ibc[:, 0:1], scalar2=None, op0=ALU.add,
                )
                nc.vector.tensor_scalar(
                    out=idx16[:, 2:3], in0=poff[:, 2:3],
                    scalar1=ibc[:, 0:1], scalar2=0.0625,
                    op0=ALU.add, op1=ALU.mult,
                )

                # gpsimd wake-up: a dependent no-op so the engine is
                # actively polling when idx16 lands
                nc.gpsimd.tensor_copy(gdum, mrow)
                # ---- select expert: ONE gpsimd ap_gather (48 idxs) ----
                nc.gpsimd.ap_gather(
                    usel3, u3_all, idx16,
                    channels=HALF, num_elems=165, d=16, num_idxs=48,
                )
                nc.vector.tensor_copy(bself, usel3[:, 512:516])

                # junk bridge B: index matmul -> first D matmul
                for j in range(N_JUNK_B):
                    jnk = jps.tile([E, 512], F32, tag="j")
                    nc.tensor.matmul(
                        jnk, lhsT=wg[:, 1, :],
                        rhs=xb[:, 1, (j % 8) * 512 : (j % 8) * 512 + 512],
                    )

            uDr = usel3[:, 0:HALF]           # (I+Wrgb)^T
            uDt = usel3[:, HALF : 2 * HALF]  # (I+Wtir)^T
            uVr = usel3[:, 256 : 256 + QUARTER]
            uVt = usel3[:, 256 + QUARTER : 256 + HALF]
            uW2 = usel3[:, 384:512]          # Wt2 replicated
            bsel = bself

            # ---- phase 2: selected expert, software-pipelined chunks ----
            with (
                tc.tile_pool(name="dps", bufs=2, space="PSUM") as dps,
                tc.tile_pool(name="hps", bufs=2, space="PSUM") as hps,
                tc.tile_pool(name="aps", bufs=2, space="PSUM") as aps,
            ):
                nch = len(CHUNKS)
                hp = [None] * nch
                hsb = [None] * nch

                def emit_h(ci):
                    off, n = CHUNKS[ci]
                    hp[ci] = hps.tile([HALF, 512], F32, tag="h", name="hp")
                    nc.tensor.matmul(
                        hp[ci][0:QUARTER, 0:n],
                        lhsT=uVr,
                        rhs=xb[:, 0, off : off + n],
                    )
                    nc.tensor.matmul(
                        hp[ci][QUARTER:HALF, 0:n],
                        lhsT=uVt,
                        rhs=xb[:, 1, off : off + n],
                        tile_position=(0, QUARTER),
                    )

                def emit_sig(ci):
                    # A layer: both halves into one paired 2-bank PSUM tile,
                    # then ONE sigmoid over both halves (S)
                    off, n = CHUNKS[ci]
                    a2 = aps.tile([HALF, 2, 512], F32, tag="a", name="a2")
                    nc.tensor.matmul(
                        a2[:, 0, 0:n],
                        lhsT=uW2[0:QUARTER, :],
                        rhs=hsb[ci][0:QUARTER, 0:n],
                        tile_position=(0, 0),
                    )
                    nc.tensor.matmul(
                        a2[:, 1, 0:n],
                        lhsT=uW2[QUARTER:HALF, :],
                        rhs=hsb[ci][QUARTER:HALF, 0:n],
                        tile_position=(QUARTER, 0),
                    )
                    sst = sstp.tile([HALF, 2, 512], BF16, tag="s", name="sst")
                    nc.scalar.activation(
                        out=sst[:, :, 0:n], in_=a2[:, :, 0:n],
                        func=AF.Sigmoid, bias=bsel[:, 3:4],
                    )
                    return sst

                def emit_comb(ci, sst, ocp, j, vadd=False):
                    # D late (short PSUM residency); (D+b)*s fused on V;
                    # final add on G into the paired out tile
                    off, n = CHUNKS[ci]
                    dr = dps.tile([HALF, 512], F32, tag="d", name="dr")
                    nc.tensor.matmul(
                        dr[:, 0:n], lhsT=uDr, rhs=xb[:, 0, off : off + n]
                    )
                    dt = dps.tile([HALF, 512], F32, tag="d", name="dt")
                    nc.tensor.matmul(
                        dt[:, 0:n], lhsT=uDt, rhs=xb[:, 1, off : off + n]
                    )
                    prt = combp.tile([HALF, 512], BF16, tag="p", name="prt")
                    nc.vector.scalar_tensor_tensor(
                        out=prt[:, 0:n], in0=dr[:, 0:n], scalar=bsel[:, 0:1],
                        in1=sst[:, 0, 0:n], op0=ALU.add, op1=ALU.mult,
                    )
                    ob = combp.tile([HALF, 512], BF16, tag="o", name="ob")
                    nc.vector.scalar_tensor_tensor(
                        out=ob[:, 0:n], in0=dt[:, 0:n], scalar=bsel[:, 1:2],
                        in1=sst[:, 1, 0:n], op0=ALU.add, op1=ALU.mult,
                    )
                    eng = nc.vector if vadd else nc.gpsimd
                    eng.tensor_add(ocp[:, j, 0:n], prt[:, 0:n], ob[:, 0:n])

                def emit_relu(ci):
                    off, n = CHUNKS[ci]
                    hsb[ci] = hsbp.tile(
                        [HALF, 512], BF16, tag="hsb", name="hsb"
                    )
                    nc.scalar.activation(
                        out=hsb[ci][:, 0:n], in_=hp[ci][:, 0:n],
                        func=AF.Relu, bias=bsel[:, 2:3],
                    )

                # single-chunk software pipeline; S queue alternates
                # [sig(k), relu(k+2)] so the A-matmul round-trip latency
                # hides under the previous sigmoid
                emit_h(0)
                if nch > 1:
                    emit_h(1)
                emit_relu(0)
                if nch > 1:
                    emit_relu(1)
                ocp = None
                for k in range(nch):
                    sst = emit_sig(k)
                    if k + 2 < nch:
                        emit_h(k + 2)
                        emit_relu(k + 2)
                    if k % 2 == 0:
                        ocp = combp.tile(
                            [HALF, 2, 512], BF16, tag="c", name="ocp"
                        )
                    emit_comb(k, sst, ocp, k % 2)
                    if k % 2 == 1 or k == nch - 1:
                        base = k - (k % 2)
                        off0 = CHUNKS[base][0]
                        ntot = sum(CHUNKS[c][1] for c in range(base, k + 1))
                        oq = nc.sync if (base // 2) % 2 == 0 else nc.scalar
                        oq.dma_start(
                            out=out_d[:, off0 : off0 + ntot],
                            in_=ocp[:, 0:2, 0:512]
                            if ntot == 1024
                            else ocp[:, 0, 0:ntot],
                        )

    nc.compile()
    return nc


def _pack_inputs(x, Wg, bg, Wrgb, brgb, Wtir, btir, Wt1, bt1, Wt2, bt2):
    import ml_dtypes
    eye = np.eye(HALF, dtype=np.float32)
    # per-expert 256-wide payloads, then sliced into 16 rows of 16
    udp = np.zeros((E, HALF, 256), dtype=np.float32)
    uhp = np.zeros((E, HALF, 256), dtype=np.float32)
    for e in range(E):
        Ar = Wrgb[e] + eye                      # [o, c]
        At = Wtir[e] + eye
        udp[e, :, 0:HALF] = Ar.T
        udp[e, :, HALF:] = At.T
        uhp[e, :, 0:QUARTER] = (Wt1[e] @ Ar).T
        uhp[e, :, QUARTER:HALF] = (Wt1[e] @ At).T
        rep = np.repeat(Wt2[e, 0][:, None], HALF, axis=1)   # [64, 128]
        uhp[e, 0:QUARTER, HALF:] = rep
        uhp[e, QUARTER:HALF, HALF:] = rep

    def to_table(p):
        # [E, 128, 256] -> [128, 16E, 16]: row 16e+j = payload cols 16j:16j+16
        t = p.reshape(E, HALF, 16, 16).transpose(1, 0, 2, 3).reshape(
            HALF, 16 * E, 16
        )
        return np.ascontiguousarray(t).astype(ml_dtypes.bfloat16)

    u3t = np.concatenate([to_table(udp), to_table(uhp)], axis=1)

    # bias rows 160+e of the fused table: [b_rgb, b_tir, c_stack, bt2, 0..]
    bias_t = np.zeros((HALF, E, 16), dtype=np.float32)
    for e in range(E):
        bias_t[:, e, 0] = brgb[e]
        bias_t[:, e, 1] = btir[e]
        bias_t[0:QUARTER, e, 2] = bt1[e] + Wt1[e] @ brgb[e]
        bias_t[QUARTER:HALF, e, 2] = bt1[e] + Wt1[e] @ btir[e]
        bias_t[:, e, 3] = bt2[e, 0]
    u3 = np.ascontiguousarray(np.concatenate(
        [u3t, bias_t.astype(ml_dtypes.bfloat16)], axis=1
    ))

    wgt = Wg.T.astype(np.float32)                   # [256, 5]
    wg_p = np.ascontiguousarray(
        np.stack([wgt[:HALF], wgt[HALF:]], axis=1)
    ).astype(ml_dtypes.bfloat16)                    # [128, 2, 5]
    bgx = np.ascontiguousarray((bg * float(HW))[None, :].astype(np.float32))
    # first table-row index of expert e
    ivx = (16.0 * np.arange(E, dtype=np.float32))[None, :]
    # wrapped per-16-partition row indices: partition p gathers rows
    # 16e+p%16 (uD), 80+16e+p%16 (uHA), 160+e (bias; (2560+16e)/16)
    pmod = np.arange(HALF, dtype=np.float32) % 16
    poffx = np.stack(
        [pmod, pmod + 16.0 * E, np.full(HALF, 2560.0, np.float32)], axis=1
    ).copy()

    common = {"u3": u3, "wg": wg_p, "bg": bgx, "iv": ivx, "poff": poffx}
    in_maps = []
    for b in range(B):
        m = dict(common)
        xr = x[b].reshape(C, HW)
        m["x0"] = np.ascontiguousarray(xr[:HALF]).astype(ml_dtypes.bfloat16)
        m["x1"] = np.ascontiguousarray(xr[HALF:]).astype(ml_dtypes.bfloat16)
        in_maps.append(m)
    return in_maps


_NC_CACHE = {}


def _get_nc():
    if "nc" not in _NC_CACHE:
        _NC_CACHE["nc"] = build_nc()
    return _NC_CACHE["nc"]


def kernel(x, Wg, bg, Wrgb, brgb, Wtir, btir, Wt1, bt1, Wt2, bt2, **run_kw):
    nc = _get_nc()
    in_maps = _pack_inputs(
        np.asarray(x), np.asarray(Wg), np.asarray(bg), np.asarray(Wrgb),
        np.asarray(brgb), np.asarray(Wtir), np.asarray(btir),
        np.asarray(Wt1), np.asarray(bt1), np.asarray(Wt2), np.asarray(bt2),
    )
    res = run_bass_kernel_spmd(nc, in_maps, core_ids=list(range(NCORES)), **run_kw)
    out = np.stack(
        [np.asarray(r["out"]).astype(np.float32) for r in res.results], axis=0
    )
    if run_kw:
        kernel.last_results = res
    return out.reshape(B, HALF, H, W)
